# revision 33
# baseline (speedup 1.0000x reference)
"""Trainium2 Bass kernel for nn_Double_SSM_Block_Encoder.

Double Mamba (SSM) block encoder over (b=8, c=64, h=64, w=64) inputs.
Sharding: data-parallel over batch, 1 batch element per NeuronCore (8 cores).

Per-core layout: channel-major [channels on partitions, time t = h*64+w on free].
Architecture (vs the v1 baseline):
  - B/C state rows broadcast to 128 partitions via stride-0 DMA from a DRAM
    scratch copy of proj (replaces PE ones-matmuls + PSUM->SBUF scalar copies)
  - y = D*xc + sum_n C_n*h_n accumulated on the idle PE: diag(D) @ xc seeds a
    PSUM accumulation chain, then I @ (h_n * Cb_n) per state; zs-gating fused
    on PSUM copy-out (no DVE adds at all for the state reduction)
  - layernorm mean folded into the output weights on host
    (wout_c = wout - colmean(wout) => y1 comes out centered; var = mean(y1c^2))
  - chunk-outer stage 5 (2048-superchunks) with carry columns so the per-state
    scan chains across chunks with only 3 h buffers resident
  - the two mamba blocks' pipelines interleave at emission (block2's front
    stages run between block1's scan chunks) to keep the DVE dense
  - dt/operands in fp16 so DVE tensor_tensor muls run in 2x mode
The sequential scan itself (tensor_tensor_scan, ~2.2ns/elem) is the DVE
throughput floor: 16 states x 4096 steps x 2 blocks ~= 284us of the ~540us.
"""
import sys, types, contextlib, ctypes
sys.path.insert(0, "/opt/trn_rl_repo")
import numpy as np

# ---- axon NTFF profile hook shim (image's antenv lacks axon_hooks) ----------
def _make_ntff_hook(so_path="/opt/axon/libaxon_pjrt.so"):
    try:
        lib = ctypes.CDLL(so_path)
    except OSError:
        return None
    if not hasattr(lib, "axon_start_nrt_profile"):
        return None
    lib.axon_start_nrt_profile.argtypes = [ctypes.POINTER(ctypes.c_int64), ctypes.c_size_t]
    lib.axon_start_nrt_profile.restype = ctypes.c_int64
    lib.axon_stop_nrt_profile.argtypes = [ctypes.c_char_p]
    lib.axon_stop_nrt_profile.restype = ctypes.c_int64

    @contextlib.contextmanager
    def _hook(output_dir, device_ids):
        import jax
        jax.devices()
        if device_ids:
            ids = (ctypes.c_int64 * len(device_ids))(*device_ids)
            rc = lib.axon_start_nrt_profile(ids, len(device_ids))
        else:
            rc = lib.axon_start_nrt_profile(None, 0)
        if rc != 0:
            raise RuntimeError(f"axon_start_nrt_profile rc={rc}")
        try:
            yield
        finally:
            rc = lib.axon_stop_nrt_profile(str(output_dir).encode())
            if rc != 0:
                print(f"WARNING: axon_stop_nrt_profile rc={rc} (no NTFF shipped)")
    return _hook

if "antenv.axon_hooks" not in sys.modules:
    _hooks_mod = types.ModuleType("antenv.axon_hooks")
    _HOOK = _make_ntff_hook()
    _hooks_mod.get_axon_ntff_profile_hook = lambda: _HOOK
    _hooks_mod.set_axon_ntff_profile_hook = lambda h: None
    sys.modules["antenv.axon_hooks"] = _hooks_mod

import concourse.bass as bass
import concourse.tile as tile
from concourse import mybir
from concourse import bass_utils
from bass_rust import AP as RustAP
bass_utils.upload_artifacts = lambda tmpdir: tmpdir  # no S3 in this container
from contextlib import ExitStack

F32 = mybir.dt.float32
F16 = mybir.dt.float16
AF = mybir.ActivationFunctionType
OP = mybir.AluOpType

NCORES = 8
CIN = 64        # model channels in
D = 128         # d_inner
NST = 16        # d_state
RANK = 4        # dt_rank
KCONV = 4
L = 4096
TM = 512        # matmul free-dim tile (ISA limit)
T = 1024        # activation / PSUM tile (2 matmuls per PSUM tile)
NT = L // T
SCH = 2048      # stage-5 superchunk (scan granularity; PE y-accum per 1024)


def _legalize_sync_waits(nc):
    """Walrus codegen allows only one inline sync-wait per compute
    instruction; hoist surplus waits onto a preceding same-engine Drain."""
    SAFE = set()
    for f in nc.m.functions:
        for blk in f.blocks:
            insts = blk.instructions
            i = 0
            while i < len(insts):
                inst = insts[i]
                si = inst.sync_info
                if (si is not None and si.on_wait and len(si.on_wait) > 1
                        and inst.opcode not in SAFE):
                    waits = list(si.on_wait)
                    for w in waits[:-1]:
                        d = mybir.InstDrain(
                            name=nc.get_next_instruction_name(),
                            ins=[], outs=[], bass_is_fusable=False)
                        d.engine = inst.engine
                        d.sync_info = mybir.SyncInfo(on_wait=[w], on_update=[])
                        insts.insert(i, d)
                        i += 1
                    inst.sync_info = mybir.SyncInfo(
                        on_wait=[waits[-1]], on_update=list(si.on_update))
                    i += 1
                else:
                    i += 1


SIM_SAFE = False  # emit Silu as Identity+Sigmoid+mul so CoreSim can run it


def _emit_silu(nc, nlp, out_sl, in_ps, bias, blk_i, j, which):
    if not SIM_SAFE:
        if bias is None:
            nc.scalar.activation(out_sl, in_ps, AF.Silu)
        else:
            nc.scalar.activation(out_sl, in_ps, AF.Silu, bias=bias)
        return
    v = nlp.tile(list(in_ps.shape), F32, tag="lnt", name=f"sv_{which}_{blk_i}_{j}")
    if bias is None:
        nc.scalar.activation(v[:], in_ps, AF.Identity)
    else:
        nc.scalar.activation(v[:], in_ps, AF.Identity, bias=bias)
    s = nlp.tile(list(in_ps.shape), F32, tag="lnt2", name=f"ss_{which}_{blk_i}_{j}")
    nc.scalar.activation(s[:], v[:], AF.Sigmoid)
    nc.vector.tensor_mul(out_sl, v[:], s[:])


def _bcast_ap(dram_row):
    """DRAM [1, n] row -> AP replicating it across 128 partitions."""
    return dram_row.partition_broadcast(128)


class _Block:
    """Per-block emission helpers so the two blocks' pipelines can interleave.

    front(j): stages 1-4 for time-tile j (xz matmuls+silu, proj, softplus dt,
              dtxc) — emitted per tile so ACT work pipelines chunk-wise.
    scan_chunk(c): stage-5 superchunk (16 states: dA, dbx, scan, hC, PE
              y-accumulation in PSUM, fused zs-mul on copy-out).
    stage7(j): out matmul + layernorm + relu for tile j (writes x2pad or the
              permuted final output).
    """

    def __init__(self, nc, pools, P, projd, blk_i, xpad, out_final):
        self.nc, self.P, self.projd = nc, P, projd
        self.blk_i, self.xpad, self.out_final = blk_i, xpad, out_final
        (self.const, self.big, self.nlp, self.nlp2, self.hpool,
         self.psA, self.psP, self.psY0, self.psY1) = pools
        self.COUT = P["wout"].shape[1]
        s = f"_{blk_i}"
        self.xc = self.big.tile([D, L], F16, tag="xc"+s, name="xc"+s)
        self.zs = self.big.tile([D, L], F16, tag="zs"+s, name="zs"+s)
        self.proj = self.big.tile([RANK + 2*NST, L], F16, tag="proj"+s, name="proj"+s)
        self.dt = self.big.tile([D, L], F16, tag="dt"+s, name="dt"+s)
        self.dtxc = self.big.tile([D, L], F16, tag="dtxc"+s, name="dtxc"+s)
        self.y_sb = self.big.tile([D, L], F16, tag="ysb", name="ysb"+s)
        self.carry = self.big.tile([D, NST], F16, tag="carry", name="carry"+s)
        self.musq = self.big.tile([1, L], F16, tag="musq", name="musq"+s)
        if out_final is None:
            self.x2pad = self.big.tile([self.COUT, 3 + L], F16, tag="xpad",
                                       name="x2pad"+s)
            nc.vector.memset(self.x2pad[:, 0:3], 0.0)

    def front(self, j):
        nc, P = self.nc, self.P
        b, sl = self.blk_i, slice(j*T, (j+1)*T)
        ps_xc = self.psA.tile([D, T], F32, tag="mm", name=f"psxc_{b}_{j}")
        for h in range(T // TM):
            t0 = j*T + h*TM
            for k in range(KCONV):
                nc.tensor.matmul(ps_xc[:, h*TM:(h+1)*TM], P["wk"][k][:],
                                 self.xpad[:, t0 + k : t0 + k + TM],
                                 start=(k == 0), stop=(k == KCONV - 1))
        _emit_silu(nc, self.nlp, self.xc[:, sl], ps_xc[:], P["bconv"][:], b, j, "xc")
        ps_z = self.psA.tile([D, T], F32, tag="mm", name=f"psz_{b}_{j}")
        for h in range(T // TM):
            t0 = j*T + h*TM
            nc.tensor.matmul(ps_z[:, h*TM:(h+1)*TM], P["wz"][:],
                             self.xpad[:, 3 + t0 : 3 + t0 + TM],
                             start=True, stop=True)
        _emit_silu(nc, self.nlp, self.zs[:, sl], ps_z[:], None, b, j, "z")
        ps_p = self.psP.tile([RANK + 2*NST, T], F32, tag="pp", name=f"psp_{b}_{j}")
        for h in range(T // TM):
            nc.tensor.matmul(ps_p[:, h*TM:(h+1)*TM], P["wx"][:],
                             self.xc[:, j*T + h*TM : j*T + (h+1)*TM],
                             start=True, stop=True)
        nc.scalar.copy(self.proj[:, sl], ps_p[:])
        nc.sync.dma_start(self.projd[:, sl], self.proj[RANK:, sl])
        ps_d = self.psA.tile([D, T], F32, tag="mm", name=f"psd_{b}_{j}")
        for h in range(T // TM):
            nc.tensor.matmul(ps_d[:, h*TM:(h+1)*TM], P["wdt"][:],
                             self.proj[0:RANK, j*T + h*TM : j*T + (h+1)*TM],
                             start=True, stop=True)
        e_t = self.nlp.tile([D, T], F16, tag="lnt", name=f"spe_{b}_{j}")
        nc.scalar.activation(e_t[:], ps_d[:], AF.Exp, bias=P["bdt"][:])
        nc.scalar.activation(self.dt[:, sl], e_t[:], AF.Ln, bias=self.const["one_d"][:])
        nc.vector.tensor_mul(self.dtxc[:, sl], self.dt[:, sl], self.xc[:, sl])

    def scan_chunk(self, c, mid_cb=None, c0=None, SCH=SCH, first=False):
        nc, P = self.nc, self.P
        b = self.blk_i
        if c0 is None:
            c0 = c * SCH
        NCHK = T // TM  # PSUM 1024-tiles per superchunk half pair
        NH = SCH // T   # number of 1024-halves
        pools_y = [self.psY0, self.psY1]
        y_ps = [pools_y[p % 2].tile([D, T], F32, tag=f"yps{p % 2}",
                                    name=f"yps{p}_{b}_{c}") for p in range(NH)]
        for p in range(NH):
            for s in range(T // TM):
                o = c0 + p*T + s*TM
                nc.tensor.matmul(y_ps[p][:, s*TM:(s+1)*TM], P["diagD"][:],
                                 self.xc[:, o:o+TM], start=True, stop=False)
        for n in range(NST):
            bc = self.nlp2.tile([D, 2, SCH], F16, tag="bc", bufs=3,
                                name=f"bc_{b}_{n}_{c}")
            b0 = self.projd[n:n+1, c0:c0+SCH]
            bc_src = RustAP(b0.tensor, b0.offset, [[0, 128], [NST * L, 2], [1, SCH]])
            nc.sync.dma_start(bc[:], bc_src)
            dA = self.nlp2.tile([D, SCH], F16, tag="dA", bufs=3,
                                name=f"dA_{b}_{n}_{c}")
            nc.scalar.activation(dA[:], self.dt[:, c0:c0+SCH], AF.Exp,
                                 scale=P["A"][:, n:n+1])
            dbx = self.nlp2.tile([D, SCH], F16, tag="dbx", bufs=3,
                                 name=f"dbx_{b}_{n}_{c}")
            nc.vector.tensor_mul(dbx[:], self.dtxc[:, c0:c0+SCH], bc[:, 0, :])
            h = self.hpool.tile([D, SCH], F16, tag="h", bufs=3,
                                name=f"h_{b}_{n}_{c}")
            init = 0.0 if first else self.carry[:, n:n+1]
            nc.vector.tensor_tensor_scan(h[:], dA[:], dbx[:], init,
                                         OP.mult, OP.add)
            if c0 + SCH < L:
                nc.scalar.copy(self.carry[:, n:n+1], h[:, SCH-1:SCH])
            hC = self.nlp2.tile([D, SCH], F16, tag="hC", bufs=3,
                                name=f"hC_{b}_{n}_{c}")
            nc.vector.tensor_mul(hC[:], h[:], bc[:, 1, :])
            for p in range(NH):
                for s in range(T // TM):
                    o = p*T + s*TM
                    nc.tensor.matmul(y_ps[p][:, s*TM:(s+1)*TM], P["ident"][:],
                                     hC[:, o:o+TM],
                                     start=False, stop=(n == NST - 1))
            if n == 1 and mid_cb is not None:
                mid_cb()
        for p in range(NH):
            sl = slice(c0 + p*T, c0 + (p+1)*T)
            nc.scalar.copy(self.y_sb[:, sl], y_ps[p][:])
            nc.vector.tensor_mul(self.y_sb[:, sl], self.y_sb[:, sl], self.zs[:, sl])

    def stage7(self, j, psy=None, psr=None):
        """Out-projection + layernorm + relu for tile j.

        wout is pre-centered on host (wout - colmean(wout)) so the matmul
        yields y1c = y1 - mean_c(y1) directly; var = mean_c(y1c^2).
        psy/psr override the PSUM pools (tail tiles use the freed psY banks)."""
        nc, P, COUT = self.nc, self.P, self.COUT
        b, sl = self.blk_i, slice(j*T, (j+1)*T)
        ptag = "mm" if psy is None else ("yps0" if psy is self.psY0 else "yps1")
        y1 = self.nlp.tile([COUT, T], F16, tag="y1", bufs=2, name=f"y1_{b}_{j}")
        ps_y = (psy or self.psA).tile([COUT, T], F32, tag=ptag,
                                      name=f"psy_{b}_{j}")
        for h in range(T // TM):
            nc.tensor.matmul(ps_y[:, h*TM:(h+1)*TM], P["wout"][:],
                             self.y_sb[:, j*T + h*TM : j*T + (h+1)*TM],
                             start=True, stop=True)
        nc.scalar.copy(y1[:], ps_y[:])
        y1sq = self.nlp.tile([COUT, T], F16, tag="lnt", name=f"y1sq_{b}_{j}")
        nc.scalar.activation(y1sq[:], y1[:], AF.Square)
        ps_m2 = self.psP.tile([1, T], F32, tag="pp", name=f"psm2_{b}_{j}")
        for h in range(T // TM):
            nc.tensor.matmul(ps_m2[:, h*TM:(h+1)*TM], P["onesc"][:],
                             y1sq[:, h*TM:(h+1)*TM], start=True, stop=True)
        nc.scalar.copy(self.musq[:, sl], ps_m2[:])
        nc.scalar.activation(self.musq[:, sl], self.musq[:, sl], AF.Ln,
                             bias=self.const["eps"][:])
        nc.scalar.activation(self.musq[:, sl], self.musq[:, sl], AF.Exp, scale=-0.5)
        ps_rb = (psr or self.psA).tile([COUT, T], F32, tag=ptag,
                                       name=f"psrb_{b}_{j}")
        for h in range(T // TM):
            nc.tensor.matmul(ps_rb[:, h*TM:(h+1)*TM], P["onesr"][:],
                             self.musq[:, j*T + h*TM : j*T + (h+1)*TM],
                             start=True, stop=True)
        rb = self.nlp.tile([COUT, T], F16, tag="lnt", name=f"rb_{b}_{j}")
        nc.scalar.copy(rb[:], ps_rb[:])
        t2 = self.nlp.tile([COUT, T], F16, tag="lnt2", name=f"lnt2_{b}_{j}")
        nc.vector.tensor_mul(t2[:], y1[:], rb[:])
        if self.out_final is None:
            nc.scalar.activation(self.x2pad[:, 3 + j*T : 3 + (j+1)*T], t2[:],
                                 AF.Relu, bias=P["bln"][:], scale=P["gln"][:])
        else:
            in_v = t2[:].rearrange("p (h w) -> p h w", w=64)
            out_v = self.out_final[:].rearrange("p (w h) -> p h w", h=64)[:, 16*j:16*(j+1), :]
            nc.scalar.activation(out_v, in_v, AF.Relu,
                                 bias=P["bln"][:], scale=P["gln"][:])


    def stage7_pair(self, ja, jb):
        """stage7 for two tiles with steps interleaved so the per-step ACT/PE
        latencies of the two chains pipeline (used for the kernel tail).
        Uses psY0 for ja's PSUM tiles and psY1 for jb's (free after the last
        scan chunk)."""
        nc, P, COUT = self.nc, self.P, self.COUT
        b = self.blk_i
        js = (ja, jb)
        pools_y = {ja: (self.psY0, "yps0"), jb: (self.psY1, "yps1")}
        y1_, psy_, sq_, psm_, psr_, rb_, t2_ = {}, {}, {}, {}, {}, {}, {}
        for j in js:
            pool, tag = pools_y[j]
            y1_[j] = self.nlp.tile([COUT, T], F16, tag="y1", bufs=2,
                                   name=f"y1_{b}_{j}")
            psy_[j] = pool.tile([COUT, T], F32, tag=tag, name=f"psy_{b}_{j}")
            for h in range(T // TM):
                nc.tensor.matmul(psy_[j][:, h*TM:(h+1)*TM], P["wout"][:],
                                 self.y_sb[:, j*T + h*TM : j*T + (h+1)*TM],
                                 start=True, stop=True)
        for j in js:
            nc.scalar.copy(y1_[j][:], psy_[j][:])
        for j in js:
            sq_[j] = self.nlp.tile([COUT, T], F16, tag="lnt",
                                   name=f"y1sq_{b}_{j}")
            nc.scalar.activation(sq_[j][:], y1_[j][:], AF.Square)
        for j in js:
            psm_[j] = self.psP.tile([1, T], F32, tag="pp", name=f"psm2_{b}_{j}")
            for h in range(T // TM):
                nc.tensor.matmul(psm_[j][:, h*TM:(h+1)*TM], P["onesc"][:],
                                 sq_[j][:, h*TM:(h+1)*TM], start=True, stop=True)
            sl = slice(j*T, (j+1)*T)
            nc.scalar.copy(self.musq[:, sl], psm_[j][:])
        for j in js:
            sl = slice(j*T, (j+1)*T)
            nc.scalar.activation(self.musq[:, sl], self.musq[:, sl], AF.Ln,
                                 bias=self.const["eps"][:])
        for j in js:
            sl = slice(j*T, (j+1)*T)
            nc.scalar.activation(self.musq[:, sl], self.musq[:, sl], AF.Exp,
                                 scale=-0.5)
        for j in js:
            pool, tag = pools_y[j]
            psr_[j] = pool.tile([COUT, T], F32, tag=tag, name=f"psrb_{b}_{j}")
            for h in range(T // TM):
                nc.tensor.matmul(psr_[j][:, h*TM:(h+1)*TM], P["onesr"][:],
                                 self.musq[:, j*T + h*TM : j*T + (h+1)*TM],
                                 start=True, stop=True)
        for j in js:
            rb_[j] = self.nlp.tile([COUT, T], F16, tag="lnt", name=f"rb_{b}_{j}")
            nc.scalar.copy(rb_[j][:], psr_[j][:])
        for j in js:
            t2_[j] = self.nlp.tile([COUT, T], F16, tag="lnt2",
                                   name=f"lnt2_{b}_{j}")
            nc.vector.tensor_mul(t2_[j][:], y1_[j][:], rb_[j][:])
        for j in js:
            if self.out_final is None:
                nc.scalar.activation(self.x2pad[:, 3 + j*T : 3 + (j+1)*T],
                                     t2_[j][:], AF.Relu,
                                     bias=P["bln"][:], scale=P["gln"][:])
            else:
                in_v = t2_[j][:].rearrange("p (h w) -> p h w", w=64)
                out_v = self.out_final[:].rearrange("p (w h) -> p h w", h=64)[:, 16*j:16*(j+1), :]
                nc.scalar.activation(out_v, in_v, AF.Relu,
                                     bias=P["bln"][:], scale=P["gln"][:])


def build_nc(legalize=True, sim_safe=False):
    global SIM_SAFE
    SIM_SAFE = sim_safe
    nc = bass.Bass("TRN2", debug=False)

    def din(name, shape, dt=F32):
        return nc.dram_tensor(name, list(shape), dt, kind="ExternalInput")

    x_d = din("x", (CIN, L), F16)
    ins = {}
    for b in (1, 2):
        ins[f"wk{b}"] = [din(f"wk{b}_{k}", (CIN, D), F16) for k in range(KCONV)]
        ins[f"wz{b}"] = din(f"wz{b}", (CIN, D), F16)
        ins[f"bconv{b}"] = din(f"bconv{b}", (D, 1))
        ins[f"wx{b}"] = din(f"wx{b}", (D, RANK + 2*NST), F16)
        ins[f"wdt{b}"] = din(f"wdt{b}", (RANK, D), F16)
        ins[f"bdt{b}"] = din(f"bdt{b}", (D, 1))
        ins[f"A{b}"] = din(f"A{b}", (D, NST))
        ins[f"D{b}"] = din(f"D{b}", (D, 1))
        cout = CIN if b == 1 else 2 * CIN
        ins[f"wout{b}"] = din(f"wout{b}", (D, cout), F16)
        ins[f"gln{b}"] = din(f"gln{b}", (cout, 1))
        ins[f"bln{b}"] = din(f"bln{b}", (cout, 1))
        ins[f"onesc{b}"] = din(f"onesc{b}", (cout, 1), F16)   # 1/cout for mean
        ins[f"onesr{b}"] = din(f"onesr{b}", (1, cout), F16)   # ones row for bcast
        ins[f"diagD{b}"] = din(f"diagD{b}", (D, D), F16)      # diag(D) for PE y-init
    ins["one_d"] = din("one_d", (D, 1))
    ins["ident"] = din("ident", (D, D), F16)
    ins["eps"] = din("eps", (1, 1))
    out_d = nc.dram_tensor("out", [2*CIN, L], F16, kind="ExternalOutput")

    with tile.TileContext(nc) as tc:
        with ExitStack() as ctx:
            cpool = ctx.enter_context(tc.tile_pool(name="const", bufs=1))
            big = ctx.enter_context(tc.tile_pool(name="big", bufs=1))
            nlp = ctx.enter_context(tc.tile_pool(name="nloop", bufs=2))
            nlp2 = ctx.enter_context(tc.tile_pool(name="nloop2", bufs=2))
            hpool = ctx.enter_context(tc.tile_pool(name="hpool", bufs=20))
            dramp = ctx.enter_context(tc.tile_pool(name="dram", bufs=1, space="DRAM"))
            psA = ctx.enter_context(tc.tile_pool(name="psA", bufs=1, space="PSUM"))
            psP = ctx.enter_context(tc.tile_pool(name="psP", bufs=1, space="PSUM"))
            psY0 = ctx.enter_context(tc.tile_pool(name="psY0", bufs=1, space="PSUM"))
            psY1 = ctx.enter_context(tc.tile_pool(name="psY1", bufs=1, space="PSUM"))

            def load(name, dram, eng=None):
                t = cpool.tile(list(dram.shape), dram.dtype, tag=name, name=name)
                (eng or nc.gpsimd).dma_start(t[:], dram.ap())
                return t

            warm = cpool.tile([1, 2], F32, tag="warm")
            nc.vector.memset(warm[:], 0.0)
            nc.scalar.activation(warm[:, 1:2], warm[:, 0:1], AF.Silu)
            xpad = big.tile([CIN, 3 + L], F16, tag="xpad")
            nc.vector.memset(xpad[:, 0:3], 0.0)
            for _xj in range(4):
                nc.sync.dma_start(xpad[:, 3 + _xj*1024 : 3 + (_xj+1)*1024],
                                  x_d.ap()[:, _xj*1024:(_xj+1)*1024])
            const = {"one_d": load("one_d", ins["one_d"]),
                     "eps": load("eps", ins["eps"])}
            ident_t = load("ident", ins["ident"])
            P = {}
            for b in (1, 2):
                P[b] = {
                    "wk": [load(f"wk{b}_{k}", ins[f"wk{b}"][k], eng=nc.sync) for k in range(KCONV)],
                    "wz": load(f"wz{b}", ins[f"wz{b}"], eng=nc.sync),
                    "bconv": load(f"bconv{b}", ins[f"bconv{b}"], eng=nc.sync),
                    "wx": load(f"wx{b}", ins[f"wx{b}"], eng=nc.sync),
                    "wdt": load(f"wdt{b}", ins[f"wdt{b}"], eng=nc.sync),
                    "bdt": load(f"bdt{b}", ins[f"bdt{b}"], eng=nc.sync),
                    "A": load(f"A{b}", ins[f"A{b}"]),
                    "D": load(f"D{b}", ins[f"D{b}"]),
                    "wout": load(f"wout{b}", ins[f"wout{b}"]),
                    "gln": load(f"gln{b}", ins[f"gln{b}"]),
                    "bln": load(f"bln{b}", ins[f"bln{b}"]),
                    "onesc": load(f"onesc{b}", ins[f"onesc{b}"]),
                    "onesr": load(f"onesr{b}", ins[f"onesr{b}"]),
                    "diagD": load(f"diagD{b}", ins[f"diagD{b}"]),
                    "ident": ident_t,
                }

            out_sb = big.tile([2*CIN, L], F16, tag="dtxc_1")  # dtxc1 dead by then
            projd1 = dramp.tile([2*NST, L], F16, tag="projd1")
            projd2 = dramp.tile([2*NST, L], F16, tag="projd2")
            pools = (const, big, nlp, nlp2, hpool, psA, psP, psY0, psY1)
            b1 = _Block(nc, pools, P[1], projd1, 1, xpad, out_final=None)
            b1.front(0); b1.front(1)
            b2 = _Block(nc, pools, P[2], projd2, 2, b1.x2pad, out_final=out_sb)
            b1.scan_chunk(0, c0=0, SCH=1024, first=True)
            b1.scan_chunk(1, c0=1024, SCH=1024,
                          mid_cb=lambda: (b1.front(2), b1.front(3)))
            b1.stage7(0); b1.stage7(1)
            b2.front(0); b2.front(1)
            b1.scan_chunk(2, c0=2048, SCH=2048)
            b1.stage7(2); b1.stage7(3)
            b2.front(2); b2.front(3)
            b2.scan_chunk(0, first=True)
            b2.stage7(0); b2.stage7(1)
            b2.scan_chunk(1)
            b2.stage7_pair(2, 3)
            for _oj in range(4):
                nc.sync.dma_start(out_d.ap()[:, _oj*1024:(_oj+1)*1024],
                                  out_sb[:, _oj*1024:(_oj+1)*1024])

    if legalize:
        _legalize_sync_waits(nc)
    return nc


_NC_CACHE = {}
_LAST_EXEC_NS = {}

def _get_nc():
    if "nc" not in _NC_CACHE:
        _NC_CACHE["nc"] = build_nc()
    return _NC_CACHE["nc"]


def _host_params(inputs):
    """Fold conv into input projection; compute derived tensors."""
    f32 = np.float32
    maps = {}
    for b in (1, 2):
        w_in = np.asarray(inputs[f"w_in{b}"], f32)       # (64, 256)
        w_conv = np.asarray(inputs[f"w_conv{b}"], f32)   # (128, 4)
        cout = CIN if b == 1 else 2 * CIN
        for k in range(KCONV):
            maps[f"wk{b}_{k}"] = np.ascontiguousarray(w_in[:, :D] * w_conv[:, k][None, :]).astype(np.float16)
        maps[f"wz{b}"] = np.ascontiguousarray(w_in[:, D:]).astype(np.float16)
        maps[f"bconv{b}"] = np.asarray(inputs[f"b_conv{b}"], f32).reshape(D, 1)
        maps[f"wx{b}"] = np.asarray(inputs[f"w_x{b}"], np.float16)
        maps[f"wdt{b}"] = np.asarray(inputs[f"w_dt{b}"], np.float16)
        maps[f"bdt{b}"] = np.asarray(inputs[f"b_dt{b}"], f32).reshape(D, 1)
        maps[f"A{b}"] = -np.exp(np.asarray(inputs[f"A_log{b}"], f32))
        maps[f"D{b}"] = np.asarray(inputs[f"D{b}"], f32).reshape(D, 1)
        w_out = np.asarray(inputs[f"w_out{b}"], f32)
        maps[f"wout{b}"] = (w_out - w_out.mean(axis=1, keepdims=True)).astype(np.float16)
        maps[f"gln{b}"] = np.asarray(inputs[f"g_ln{b}"], f32).reshape(cout, 1)
        maps[f"bln{b}"] = np.asarray(inputs[f"b_ln{b}"], f32).reshape(cout, 1)
        maps[f"onesc{b}"] = np.full((cout, 1), 1.0 / cout, np.float16)
        maps[f"onesr{b}"] = np.ones((1, cout), np.float16)
        maps[f"diagD{b}"] = np.diag(np.asarray(inputs[f"D{b}"], f32).reshape(D)).astype(np.float16)
    maps["one_d"] = np.ones((D, 1), f32)
    maps["ident"] = np.eye(D, dtype=np.float16)
    maps["eps"] = np.full((1, 1), 1e-5, f32)
    return maps


def kernel(**inputs, ):
    return _run(inputs, trace=False)


def _run(inputs, trace=False):
    nc = _get_nc()
    x = np.asarray(inputs["x"], np.float32)              # (8, 64, 64, 64)
    b, c, hh, ww = x.shape
    params = _host_params(inputs)
    in_maps = []
    for i in range(NCORES):
        m = dict(params)
        m["x"] = np.ascontiguousarray(x[i].reshape(c, hh * ww)).astype(np.float16)
        in_maps.append(m)
    res = bass_utils.run_bass_kernel_spmd(nc, in_maps, core_ids=list(range(NCORES)),
                                          trace=trace)
    if trace:
        _LAST_EXEC_NS["ns"] = res.exec_time_ns
        _LAST_EXEC_NS["res"] = res
    out = np.stack([res.results[i]["out"] for i in range(NCORES)])
    return out.reshape(b, 2 * c, ww, hh).astype(np.float32)


# revision 34
# speedup vs baseline: 1.0143x; 1.0143x over previous
"""Trainium2 Bass kernel for nn_Double_SSM_Block_Encoder.

Double Mamba (SSM) block encoder over (b=8, c=64, h=64, w=64) inputs.
Sharding: data-parallel over batch, 1 batch element per NeuronCore (8 cores).

Per-core layout: channel-major [channels on partitions, time t = h*64+w on free].
Architecture (vs the v1 baseline):
  - B/C state rows broadcast to 128 partitions via stride-0 DMA from a DRAM
    scratch copy of proj (replaces PE ones-matmuls + PSUM->SBUF scalar copies)
  - y = D*xc + sum_n C_n*h_n accumulated on the idle PE: diag(D) @ xc seeds a
    PSUM accumulation chain, then I @ (h_n * Cb_n) per state; zs-gating fused
    on PSUM copy-out (no DVE adds at all for the state reduction)
  - layernorm mean folded into the output weights on host
    (wout_c = wout - colmean(wout) => y1 comes out centered; var = mean(y1c^2))
  - chunk-outer stage 5 (2048-superchunks) with carry columns so the per-state
    scan chains across chunks with only 3 h buffers resident
  - the two mamba blocks' pipelines interleave at emission (block2's front
    stages run between block1's scan chunks) to keep the DVE dense
  - dt/operands in fp16 so DVE tensor_tensor muls run in 2x mode
The sequential scan itself (tensor_tensor_scan, ~2.2ns/elem) is the DVE
throughput floor: 16 states x 4096 steps x 2 blocks ~= 284us of the ~540us.
"""
import sys, types, contextlib, ctypes
sys.path.insert(0, "/opt/trn_rl_repo")
import numpy as np

# ---- axon NTFF profile hook shim (image's antenv lacks axon_hooks) ----------
def _make_ntff_hook(so_path="/opt/axon/libaxon_pjrt.so"):
    try:
        lib = ctypes.CDLL(so_path)
    except OSError:
        return None
    if not hasattr(lib, "axon_start_nrt_profile"):
        return None
    lib.axon_start_nrt_profile.argtypes = [ctypes.POINTER(ctypes.c_int64), ctypes.c_size_t]
    lib.axon_start_nrt_profile.restype = ctypes.c_int64
    lib.axon_stop_nrt_profile.argtypes = [ctypes.c_char_p]
    lib.axon_stop_nrt_profile.restype = ctypes.c_int64

    @contextlib.contextmanager
    def _hook(output_dir, device_ids):
        import jax
        jax.devices()
        if device_ids:
            ids = (ctypes.c_int64 * len(device_ids))(*device_ids)
            rc = lib.axon_start_nrt_profile(ids, len(device_ids))
        else:
            rc = lib.axon_start_nrt_profile(None, 0)
        if rc != 0:
            raise RuntimeError(f"axon_start_nrt_profile rc={rc}")
        try:
            yield
        finally:
            rc = lib.axon_stop_nrt_profile(str(output_dir).encode())
            if rc != 0:
                print(f"WARNING: axon_stop_nrt_profile rc={rc} (no NTFF shipped)")
    return _hook

if "antenv.axon_hooks" not in sys.modules:
    _hooks_mod = types.ModuleType("antenv.axon_hooks")
    _HOOK = _make_ntff_hook()
    _hooks_mod.get_axon_ntff_profile_hook = lambda: _HOOK
    _hooks_mod.set_axon_ntff_profile_hook = lambda h: None
    sys.modules["antenv.axon_hooks"] = _hooks_mod

import concourse.bass as bass
import concourse.tile as tile
from concourse import mybir
from concourse import bass_utils
from bass_rust import AP as RustAP
bass_utils.upload_artifacts = lambda tmpdir: tmpdir  # no S3 in this container
from contextlib import ExitStack

F32 = mybir.dt.float32
F16 = mybir.dt.float16
AF = mybir.ActivationFunctionType
OP = mybir.AluOpType

NCORES = 8
CIN = 64        # model channels in
D = 128         # d_inner
NST = 16        # d_state
RANK = 4        # dt_rank
KCONV = 4
L = 4096
TM = 512        # matmul free-dim tile (ISA limit)
T = 1024        # activation / PSUM tile (2 matmuls per PSUM tile)
NT = L // T
SCH = 2048      # stage-5 superchunk (scan granularity; PE y-accum per 1024)


def _legalize_sync_waits(nc):
    """Walrus codegen allows only one inline sync-wait per compute
    instruction; hoist surplus waits onto a preceding same-engine Drain."""
    SAFE = set()
    for f in nc.m.functions:
        for blk in f.blocks:
            insts = blk.instructions
            i = 0
            while i < len(insts):
                inst = insts[i]
                si = inst.sync_info
                if (si is not None and si.on_wait and len(si.on_wait) > 1
                        and inst.opcode not in SAFE):
                    waits = list(si.on_wait)
                    for w in waits[:-1]:
                        d = mybir.InstDrain(
                            name=nc.get_next_instruction_name(),
                            ins=[], outs=[], bass_is_fusable=False)
                        d.engine = inst.engine
                        d.sync_info = mybir.SyncInfo(on_wait=[w], on_update=[])
                        insts.insert(i, d)
                        i += 1
                    inst.sync_info = mybir.SyncInfo(
                        on_wait=[waits[-1]], on_update=list(si.on_update))
                    i += 1
                else:
                    i += 1


SIM_SAFE = False  # emit Silu as Identity+Sigmoid+mul so CoreSim can run it


def _emit_silu(nc, nlp, out_sl, in_ps, bias, blk_i, j, which):
    if not SIM_SAFE:
        if bias is None:
            nc.scalar.activation(out_sl, in_ps, AF.Silu)
        else:
            nc.scalar.activation(out_sl, in_ps, AF.Silu, bias=bias)
        return
    v = nlp.tile(list(in_ps.shape), F32, tag="lnt", name=f"sv_{which}_{blk_i}_{j}")
    if bias is None:
        nc.scalar.activation(v[:], in_ps, AF.Identity)
    else:
        nc.scalar.activation(v[:], in_ps, AF.Identity, bias=bias)
    s = nlp.tile(list(in_ps.shape), F32, tag="lnt2", name=f"ss_{which}_{blk_i}_{j}")
    nc.scalar.activation(s[:], v[:], AF.Sigmoid)
    nc.vector.tensor_mul(out_sl, v[:], s[:])


def _bcast_ap(dram_row):
    """DRAM [1, n] row -> AP replicating it across 128 partitions."""
    return dram_row.partition_broadcast(128)


class _Block:
    """Per-block emission helpers so the two blocks' pipelines can interleave.

    front(j): stages 1-4 for time-tile j (xz matmuls+silu, proj, softplus dt,
              dtxc) — emitted per tile so ACT work pipelines chunk-wise.
    scan_chunk(c): stage-5 superchunk (16 states: dA, dbx, scan, hC, PE
              y-accumulation in PSUM, fused zs-mul on copy-out).
    stage7(j): out matmul + layernorm + relu for tile j (writes x2pad or the
              permuted final output).
    """

    def __init__(self, nc, pools, P, projd, blk_i, xpad, out_final):
        self.nc, self.P, self.projd = nc, P, projd
        self.blk_i, self.xpad, self.out_final = blk_i, xpad, out_final
        (self.const, self.big, self.nlp, self.nlp2, self.hpool,
         self.psA, self.psP, self.psY0, self.psY1) = pools
        self.COUT = P["wout"].shape[1]
        s = f"_{blk_i}"
        self.xc = self.big.tile([D, L], F16, tag="xc"+s, name="xc"+s)
        self.zs = self.big.tile([D, L], F16, tag="zs"+s, name="zs"+s)
        self.proj = self.big.tile([RANK + 2*NST, L], F16, tag="proj"+s, name="proj"+s)
        self.dt = self.big.tile([D, L], F16, tag="dt"+s, name="dt"+s)
        self.dtxc = self.big.tile([D, L], F16, tag="dtxc"+s, name="dtxc"+s)
        self.y_sb = self.big.tile([D, L], F16, tag="ysb", name="ysb"+s)
        self.carry = self.big.tile([D, NST], F16, tag="carry", name="carry"+s)
        self.musq = self.big.tile([1, L], F16, tag="musq", name="musq"+s)
        if out_final is None:
            self.x2pad = self.big.tile([self.COUT, 3 + L], F16, tag="xpad",
                                       name="x2pad"+s)
            nc.vector.memset(self.x2pad[:, 0:3], 0.0)

    def front(self, j):
        nc, P = self.nc, self.P
        b, sl = self.blk_i, slice(j*T, (j+1)*T)
        ps_xc = self.psA.tile([D, T], F32, tag="mm", name=f"psxc_{b}_{j}")
        for h in range(T // TM):
            t0 = j*T + h*TM
            for k in range(KCONV):
                nc.tensor.matmul(ps_xc[:, h*TM:(h+1)*TM], P["wk"][k][:],
                                 self.xpad[:, t0 + k : t0 + k + TM],
                                 start=(k == 0), stop=(k == KCONV - 1))
        _emit_silu(nc, self.nlp, self.xc[:, sl], ps_xc[:], P["bconv"][:], b, j, "xc")
        ps_z = self.psA.tile([D, T], F32, tag="mm", name=f"psz_{b}_{j}")
        for h in range(T // TM):
            t0 = j*T + h*TM
            nc.tensor.matmul(ps_z[:, h*TM:(h+1)*TM], P["wz"][:],
                             self.xpad[:, 3 + t0 : 3 + t0 + TM],
                             start=True, stop=True)
        _emit_silu(nc, self.nlp, self.zs[:, sl], ps_z[:], None, b, j, "z")
        ps_p = self.psP.tile([RANK + 2*NST, T], F32, tag="pp", name=f"psp_{b}_{j}")
        for h in range(T // TM):
            nc.tensor.matmul(ps_p[:, h*TM:(h+1)*TM], P["wx"][:],
                             self.xc[:, j*T + h*TM : j*T + (h+1)*TM],
                             start=True, stop=True)
        nc.scalar.copy(self.proj[:, sl], ps_p[:])
        nc.sync.dma_start(self.projd[:, sl], self.proj[RANK:, sl])
        ps_d = self.psA.tile([D, T], F32, tag="mm", name=f"psd_{b}_{j}")
        for h in range(T // TM):
            nc.tensor.matmul(ps_d[:, h*TM:(h+1)*TM], P["wdt"][:],
                             self.proj[0:RANK, j*T + h*TM : j*T + (h+1)*TM],
                             start=True, stop=True)
        e_t = self.nlp.tile([D, T], F16, tag="lnt", name=f"spe_{b}_{j}")
        nc.scalar.activation(e_t[:], ps_d[:], AF.Exp, bias=P["bdt"][:])
        nc.scalar.activation(self.dt[:, sl], e_t[:], AF.Ln, bias=self.const["one_d"][:])
        nc.vector.tensor_mul(self.dtxc[:, sl], self.dt[:, sl], self.xc[:, sl])

    def scan_chunk(self, c, mid_cb=None, c0=None, SCH=SCH, first=False):
        nc, P = self.nc, self.P
        b = self.blk_i
        if c0 is None:
            c0 = c * SCH
        NCHK = T // TM  # PSUM 1024-tiles per superchunk half pair
        NH = SCH // T   # number of 1024-halves
        pools_y = [self.psY0, self.psY1]
        y_ps = [pools_y[p % 2].tile([D, T], F32, tag=f"yps{p % 2}",
                                    name=f"yps{p}_{b}_{c}") for p in range(NH)]
        for p in range(NH):
            for s in range(T // TM):
                o = c0 + p*T + s*TM
                nc.tensor.matmul(y_ps[p][:, s*TM:(s+1)*TM], P["diagD"][:],
                                 self.xc[:, o:o+TM], start=True, stop=False)
        for n in range(NST):
            bc = self.nlp2.tile([D, 2, SCH], F16, tag="bc", bufs=4,
                                name=f"bc_{b}_{n}_{c}")
            b0 = self.projd[n:n+1, c0:c0+SCH]
            bc_src = RustAP(b0.tensor, b0.offset, [[0, 128], [NST * L, 2], [1, SCH]])
            nc.sync.dma_start(bc[:], bc_src)
            dA = self.nlp2.tile([D, SCH], F16, tag="dA", bufs=3,
                                name=f"dA_{b}_{n}_{c}")
            nc.scalar.activation(dA[:], self.dt[:, c0:c0+SCH], AF.Exp,
                                 scale=P["A"][:, n:n+1])
            dbx = self.nlp2.tile([D, SCH], F16, tag="dbx", bufs=3,
                                 name=f"dbx_{b}_{n}_{c}")
            nc.vector.tensor_mul(dbx[:], self.dtxc[:, c0:c0+SCH], bc[:, 0, :])
            h = self.hpool.tile([D, SCH], F16, tag="h", bufs=3,
                                name=f"h_{b}_{n}_{c}")
            init = 0.0 if first else self.carry[:, n:n+1]
            nc.vector.tensor_tensor_scan(h[:], dA[:], dbx[:], init,
                                         OP.mult, OP.add)
            if c0 + SCH < L:
                nc.scalar.copy(self.carry[:, n:n+1], h[:, SCH-1:SCH])
            hC = self.nlp2.tile([D, SCH], F16, tag="hC", bufs=3,
                                name=f"hC_{b}_{n}_{c}")
            nc.vector.tensor_mul(hC[:], h[:], bc[:, 1, :])
            for p in range(NH):
                for s in range(T // TM):
                    o = p*T + s*TM
                    nc.tensor.matmul(y_ps[p][:, s*TM:(s+1)*TM], P["ident"][:],
                                     hC[:, o:o+TM],
                                     start=False, stop=(n == NST - 1))
            if n == 1 and mid_cb is not None:
                mid_cb()
        for p in range(NH):
            sl = slice(c0 + p*T, c0 + (p+1)*T)
            nc.scalar.copy(self.y_sb[:, sl], y_ps[p][:])
            nc.vector.tensor_mul(self.y_sb[:, sl], self.y_sb[:, sl], self.zs[:, sl])

    def stage7(self, j, psy=None, psr=None):
        """Out-projection + layernorm + relu for tile j.

        wout is pre-centered on host (wout - colmean(wout)) so the matmul
        yields y1c = y1 - mean_c(y1) directly; var = mean_c(y1c^2).
        psy/psr override the PSUM pools (tail tiles use the freed psY banks)."""
        nc, P, COUT = self.nc, self.P, self.COUT
        b, sl = self.blk_i, slice(j*T, (j+1)*T)
        ptag = "mm" if psy is None else ("yps0" if psy is self.psY0 else "yps1")
        y1 = self.nlp.tile([COUT, T], F16, tag="y1", bufs=2, name=f"y1_{b}_{j}")
        ps_y = (psy or self.psA).tile([COUT, T], F32, tag=ptag,
                                      name=f"psy_{b}_{j}")
        for h in range(T // TM):
            nc.tensor.matmul(ps_y[:, h*TM:(h+1)*TM], P["wout"][:],
                             self.y_sb[:, j*T + h*TM : j*T + (h+1)*TM],
                             start=True, stop=True)
        nc.scalar.copy(y1[:], ps_y[:])
        y1sq = self.nlp.tile([COUT, T], F16, tag="lnt", name=f"y1sq_{b}_{j}")
        nc.scalar.activation(y1sq[:], y1[:], AF.Square)
        ps_m2 = self.psP.tile([1, T], F32, tag="pp", name=f"psm2_{b}_{j}")
        for h in range(T // TM):
            nc.tensor.matmul(ps_m2[:, h*TM:(h+1)*TM], P["onesc"][:],
                             y1sq[:, h*TM:(h+1)*TM], start=True, stop=True)
        nc.scalar.copy(self.musq[:, sl], ps_m2[:])
        nc.scalar.activation(self.musq[:, sl], self.musq[:, sl], AF.Ln,
                             bias=self.const["eps"][:])
        nc.scalar.activation(self.musq[:, sl], self.musq[:, sl], AF.Exp, scale=-0.5)
        ps_rb = (psr or self.psA).tile([COUT, T], F32, tag=ptag,
                                       name=f"psrb_{b}_{j}")
        for h in range(T // TM):
            nc.tensor.matmul(ps_rb[:, h*TM:(h+1)*TM], P["onesr"][:],
                             self.musq[:, j*T + h*TM : j*T + (h+1)*TM],
                             start=True, stop=True)
        rb = self.nlp.tile([COUT, T], F16, tag="lnt", name=f"rb_{b}_{j}")
        nc.scalar.copy(rb[:], ps_rb[:])
        t2 = self.nlp.tile([COUT, T], F16, tag="lnt2", name=f"lnt2_{b}_{j}")
        nc.vector.tensor_mul(t2[:], y1[:], rb[:])
        if self.out_final is None:
            nc.scalar.activation(self.x2pad[:, 3 + j*T : 3 + (j+1)*T], t2[:],
                                 AF.Relu, bias=P["bln"][:], scale=P["gln"][:])
        else:
            in_v = t2[:].rearrange("p (h w) -> p h w", w=64)
            out_v = self.out_final[:].rearrange("p (w h) -> p h w", h=64)[:, 16*j:16*(j+1), :]
            nc.scalar.activation(out_v, in_v, AF.Relu,
                                 bias=P["bln"][:], scale=P["gln"][:])


    def stage7_pair(self, ja, jb):
        """stage7 for two tiles with steps interleaved so the per-step ACT/PE
        latencies of the two chains pipeline (used for the kernel tail).
        Uses psY0 for ja's PSUM tiles and psY1 for jb's (free after the last
        scan chunk)."""
        nc, P, COUT = self.nc, self.P, self.COUT
        b = self.blk_i
        js = (ja, jb)
        pools_y = {ja: (self.psY0, "yps0"), jb: (self.psY1, "yps1")}
        y1_, psy_, sq_, psm_, psr_, rb_, t2_ = {}, {}, {}, {}, {}, {}, {}
        for j in js:
            pool, tag = pools_y[j]
            y1_[j] = self.nlp.tile([COUT, T], F16, tag="y1", bufs=2,
                                   name=f"y1_{b}_{j}")
            psy_[j] = pool.tile([COUT, T], F32, tag=tag, name=f"psy_{b}_{j}")
            for h in range(T // TM):
                nc.tensor.matmul(psy_[j][:, h*TM:(h+1)*TM], P["wout"][:],
                                 self.y_sb[:, j*T + h*TM : j*T + (h+1)*TM],
                                 start=True, stop=True)
        for j in js:
            nc.scalar.copy(y1_[j][:], psy_[j][:])
        for j in js:
            sq_[j] = self.nlp.tile([COUT, T], F16, tag="lnt",
                                   name=f"y1sq_{b}_{j}")
            nc.scalar.activation(sq_[j][:], y1_[j][:], AF.Square)
        for j in js:
            psm_[j] = self.psP.tile([1, T], F32, tag="pp", name=f"psm2_{b}_{j}")
            for h in range(T // TM):
                nc.tensor.matmul(psm_[j][:, h*TM:(h+1)*TM], P["onesc"][:],
                                 sq_[j][:, h*TM:(h+1)*TM], start=True, stop=True)
            sl = slice(j*T, (j+1)*T)
            nc.scalar.copy(self.musq[:, sl], psm_[j][:])
        for j in js:
            sl = slice(j*T, (j+1)*T)
            nc.scalar.activation(self.musq[:, sl], self.musq[:, sl], AF.Ln,
                                 bias=self.const["eps"][:])
        for j in js:
            sl = slice(j*T, (j+1)*T)
            nc.scalar.activation(self.musq[:, sl], self.musq[:, sl], AF.Exp,
                                 scale=-0.5)
        for j in js:
            pool, tag = pools_y[j]
            psr_[j] = pool.tile([COUT, T], F32, tag=tag, name=f"psrb_{b}_{j}")
            for h in range(T // TM):
                nc.tensor.matmul(psr_[j][:, h*TM:(h+1)*TM], P["onesr"][:],
                                 self.musq[:, j*T + h*TM : j*T + (h+1)*TM],
                                 start=True, stop=True)
        for j in js:
            rb_[j] = self.nlp.tile([COUT, T], F16, tag="lnt", name=f"rb_{b}_{j}")
            nc.scalar.copy(rb_[j][:], psr_[j][:])
        for j in js:
            t2_[j] = self.nlp.tile([COUT, T], F16, tag="lnt2",
                                   name=f"lnt2_{b}_{j}")
            nc.vector.tensor_mul(t2_[j][:], y1_[j][:], rb_[j][:])
        for j in js:
            if self.out_final is None:
                nc.scalar.activation(self.x2pad[:, 3 + j*T : 3 + (j+1)*T],
                                     t2_[j][:], AF.Relu,
                                     bias=P["bln"][:], scale=P["gln"][:])
            else:
                in_v = t2_[j][:].rearrange("p (h w) -> p h w", w=64)
                out_v = self.out_final[:].rearrange("p (w h) -> p h w", h=64)[:, 16*j:16*(j+1), :]
                nc.scalar.activation(out_v, in_v, AF.Relu,
                                     bias=P["bln"][:], scale=P["gln"][:])


def build_nc(legalize=True, sim_safe=False):
    global SIM_SAFE
    SIM_SAFE = sim_safe
    nc = bass.Bass("TRN2", debug=False)

    def din(name, shape, dt=F32):
        return nc.dram_tensor(name, list(shape), dt, kind="ExternalInput")

    x_d = din("x", (CIN, L), F16)
    ins = {}
    for b in (1, 2):
        ins[f"wk{b}"] = [din(f"wk{b}_{k}", (CIN, D), F16) for k in range(KCONV)]
        ins[f"wz{b}"] = din(f"wz{b}", (CIN, D), F16)
        ins[f"bconv{b}"] = din(f"bconv{b}", (D, 1))
        ins[f"wx{b}"] = din(f"wx{b}", (D, RANK + 2*NST), F16)
        ins[f"wdt{b}"] = din(f"wdt{b}", (RANK, D), F16)
        ins[f"bdt{b}"] = din(f"bdt{b}", (D, 1))
        ins[f"A{b}"] = din(f"A{b}", (D, NST))
        ins[f"D{b}"] = din(f"D{b}", (D, 1))
        cout = CIN if b == 1 else 2 * CIN
        ins[f"wout{b}"] = din(f"wout{b}", (D, cout), F16)
        ins[f"gln{b}"] = din(f"gln{b}", (cout, 1))
        ins[f"bln{b}"] = din(f"bln{b}", (cout, 1))
        ins[f"onesc{b}"] = din(f"onesc{b}", (cout, 1), F16)   # 1/cout for mean
        ins[f"onesr{b}"] = din(f"onesr{b}", (1, cout), F16)   # ones row for bcast
        ins[f"diagD{b}"] = din(f"diagD{b}", (D, D), F16)      # diag(D) for PE y-init
    ins["one_d"] = din("one_d", (D, 1))
    ins["ident"] = din("ident", (D, D), F16)
    ins["eps"] = din("eps", (1, 1))
    out_d = nc.dram_tensor("out", [2*CIN, L], F16, kind="ExternalOutput")

    with tile.TileContext(nc) as tc:
        with ExitStack() as ctx:
            cpool = ctx.enter_context(tc.tile_pool(name="const", bufs=1))
            big = ctx.enter_context(tc.tile_pool(name="big", bufs=1))
            nlp = ctx.enter_context(tc.tile_pool(name="nloop", bufs=2))
            nlp2 = ctx.enter_context(tc.tile_pool(name="nloop2", bufs=2))
            hpool = ctx.enter_context(tc.tile_pool(name="hpool", bufs=20))
            dramp = ctx.enter_context(tc.tile_pool(name="dram", bufs=1, space="DRAM"))
            psA = ctx.enter_context(tc.tile_pool(name="psA", bufs=1, space="PSUM"))
            psP = ctx.enter_context(tc.tile_pool(name="psP", bufs=1, space="PSUM"))
            psY0 = ctx.enter_context(tc.tile_pool(name="psY0", bufs=1, space="PSUM"))
            psY1 = ctx.enter_context(tc.tile_pool(name="psY1", bufs=1, space="PSUM"))

            def load(name, dram, eng=None):
                t = cpool.tile(list(dram.shape), dram.dtype, tag=name, name=name)
                (eng or nc.gpsimd).dma_start(t[:], dram.ap())
                return t

            warm = cpool.tile([1, 2], F32, tag="warm")
            nc.vector.memset(warm[:], 0.0)
            nc.scalar.activation(warm[:, 1:2], warm[:, 0:1], AF.Silu)
            xpad = big.tile([CIN, 3 + L], F16, tag="xpad")
            nc.vector.memset(xpad[:, 0:3], 0.0)
            for _xj in range(4):
                nc.sync.dma_start(xpad[:, 3 + _xj*1024 : 3 + (_xj+1)*1024],
                                  x_d.ap()[:, _xj*1024:(_xj+1)*1024])
            const = {"one_d": load("one_d", ins["one_d"]),
                     "eps": load("eps", ins["eps"])}
            ident_t = load("ident", ins["ident"])
            P = {}
            for b in (1, 2):
                P[b] = {
                    "wk": [load(f"wk{b}_{k}", ins[f"wk{b}"][k], eng=nc.sync) for k in range(KCONV)],
                    "wz": load(f"wz{b}", ins[f"wz{b}"], eng=nc.sync),
                    "bconv": load(f"bconv{b}", ins[f"bconv{b}"], eng=nc.sync),
                    "wx": load(f"wx{b}", ins[f"wx{b}"], eng=nc.sync),
                    "wdt": load(f"wdt{b}", ins[f"wdt{b}"], eng=nc.sync),
                    "bdt": load(f"bdt{b}", ins[f"bdt{b}"], eng=nc.sync),
                    "A": load(f"A{b}", ins[f"A{b}"]),
                    "D": load(f"D{b}", ins[f"D{b}"]),
                    "wout": load(f"wout{b}", ins[f"wout{b}"]),
                    "gln": load(f"gln{b}", ins[f"gln{b}"]),
                    "bln": load(f"bln{b}", ins[f"bln{b}"]),
                    "onesc": load(f"onesc{b}", ins[f"onesc{b}"]),
                    "onesr": load(f"onesr{b}", ins[f"onesr{b}"]),
                    "diagD": load(f"diagD{b}", ins[f"diagD{b}"]),
                    "ident": ident_t,
                }

            out_sb = big.tile([2*CIN, L], F16, tag="dtxc_1")  # dtxc1 dead by then
            projd1 = dramp.tile([2*NST, L], F16, tag="projd1")
            projd2 = dramp.tile([2*NST, L], F16, tag="projd2")
            pools = (const, big, nlp, nlp2, hpool, psA, psP, psY0, psY1)
            b1 = _Block(nc, pools, P[1], projd1, 1, xpad, out_final=None)
            b1.front(0); b1.front(1)
            b2 = _Block(nc, pools, P[2], projd2, 2, b1.x2pad, out_final=out_sb)
            b1.scan_chunk(0, mid_cb=lambda: (b1.front(2), b1.front(3)),
                          first=True)
            b1.stage7(0); b1.stage7(1)
            b2.front(0); b2.front(1)
            b1.scan_chunk(1)
            b1.stage7(2); b1.stage7(3)
            b2.front(2); b2.front(3)
            b2.scan_chunk(0, first=True)
            b2.stage7(0); b2.stage7(1)
            b2.scan_chunk(1)
            b2.stage7_pair(2, 3)
            for _oj in range(4):
                nc.sync.dma_start(out_d.ap()[:, _oj*1024:(_oj+1)*1024],
                                  out_sb[:, _oj*1024:(_oj+1)*1024])

    if legalize:
        _legalize_sync_waits(nc)
    return nc


_NC_CACHE = {}
_LAST_EXEC_NS = {}

def _get_nc():
    if "nc" not in _NC_CACHE:
        _NC_CACHE["nc"] = build_nc()
    return _NC_CACHE["nc"]


def _host_params(inputs):
    """Fold conv into input projection; compute derived tensors."""
    f32 = np.float32
    maps = {}
    for b in (1, 2):
        w_in = np.asarray(inputs[f"w_in{b}"], f32)       # (64, 256)
        w_conv = np.asarray(inputs[f"w_conv{b}"], f32)   # (128, 4)
        cout = CIN if b == 1 else 2 * CIN
        for k in range(KCONV):
            maps[f"wk{b}_{k}"] = np.ascontiguousarray(w_in[:, :D] * w_conv[:, k][None, :]).astype(np.float16)
        maps[f"wz{b}"] = np.ascontiguousarray(w_in[:, D:]).astype(np.float16)
        maps[f"bconv{b}"] = np.asarray(inputs[f"b_conv{b}"], f32).reshape(D, 1)
        maps[f"wx{b}"] = np.asarray(inputs[f"w_x{b}"], np.float16)
        maps[f"wdt{b}"] = np.asarray(inputs[f"w_dt{b}"], np.float16)
        maps[f"bdt{b}"] = np.asarray(inputs[f"b_dt{b}"], f32).reshape(D, 1)
        maps[f"A{b}"] = -np.exp(np.asarray(inputs[f"A_log{b}"], f32))
        maps[f"D{b}"] = np.asarray(inputs[f"D{b}"], f32).reshape(D, 1)
        w_out = np.asarray(inputs[f"w_out{b}"], f32)
        maps[f"wout{b}"] = (w_out - w_out.mean(axis=1, keepdims=True)).astype(np.float16)
        maps[f"gln{b}"] = np.asarray(inputs[f"g_ln{b}"], f32).reshape(cout, 1)
        maps[f"bln{b}"] = np.asarray(inputs[f"b_ln{b}"], f32).reshape(cout, 1)
        maps[f"onesc{b}"] = np.full((cout, 1), 1.0 / cout, np.float16)
        maps[f"onesr{b}"] = np.ones((1, cout), np.float16)
        maps[f"diagD{b}"] = np.diag(np.asarray(inputs[f"D{b}"], f32).reshape(D)).astype(np.float16)
    maps["one_d"] = np.ones((D, 1), f32)
    maps["ident"] = np.eye(D, dtype=np.float16)
    maps["eps"] = np.full((1, 1), 1e-5, f32)
    return maps


def kernel(**inputs, ):
    return _run(inputs, trace=False)


def _run(inputs, trace=False):
    nc = _get_nc()
    x = np.asarray(inputs["x"], np.float32)              # (8, 64, 64, 64)
    b, c, hh, ww = x.shape
    params = _host_params(inputs)
    in_maps = []
    for i in range(NCORES):
        m = dict(params)
        m["x"] = np.ascontiguousarray(x[i].reshape(c, hh * ww)).astype(np.float16)
        in_maps.append(m)
    res = bass_utils.run_bass_kernel_spmd(nc, in_maps, core_ids=list(range(NCORES)),
                                          trace=trace)
    if trace:
        _LAST_EXEC_NS["ns"] = res.exec_time_ns
        _LAST_EXEC_NS["res"] = res
    out = np.stack([res.results[i]["out"] for i in range(NCORES)])
    return out.reshape(b, 2 * c, ww, hh).astype(np.float32)
